# revision 28
# baseline (speedup 1.0000x reference)
"""Trainium2 Bass kernel for nn_CharLevelModel (token->char scatter + MLP head).

Math: reference computes
  X  = concat(h0,h1) @ W_tok + b_tok          [B,T,2D]
  tok[b,c] = last token t whose char span [lo,hi) covers c (else -1)
  G  = X[b, tok] (0 where invalid)            [B,C,2D]
  out = (G @ W1 + b1) @ W2 + b2               [B,C,2]

Everything is linear, and row-gather commutes with the per-row linear maps, so
with A = W_tok @ W1 @ W2 ([2D,2]):
  out[b,c,:] = valid * (concat[b,tok[b,c],:] @ A + b_tok@W1@W2) + (b1@W2 + b2)

On device (per core, B/8 batches; matmuls in bf16, accumulate fp32):
  mask[t,c]   = (lo[t] <= c) & (c < hi[t])
              = [ (y-lo)(y-hi) < 0 ]  with y = c+0.5   (exact integer f32 math)
              computed as  is_gt(y*s - q, y^2)  with s = lo+hi, q = lo*hi
  suffix[t,c] = sum_{t'>t} mask[t',c]       (matmul w/ triangular-ones blocks)
  sel[t,c]    = mask * (suffix == 0)        (<=1 one per column c)
  BgathT      = concat.T @ sel   via matmul(lhsT=concat_tile, rhs=sel)  [2D,C]
  outT[2,C]   = A.T @ BgathT     via matmul(lhsT=A_tile, rhs=BgathT_tile)
No transposes needed anywhere; both matmul stages consume natural layouts.
sel is exactly 0/1 (bf16-exact) so the gather matmul reproduces the bf16
hidden values exactly; only the bf16 input/A rounding matters (~1e-3 rel).

Tokens are laid out interleaved across the two 128-partition tiles:
t = 2p + ti, so each SBUF partition p holds tokens (2p, 2p+1) and the DMA from
hidden[b] ([256, 768] row-major) is a single fully-contiguous transfer.
With this order the strict "t' > t" block matrices are:
  (ti'=0,ti=0) strict   (ti'=1,ti=0) inclusive
  (ti'=0,ti=1) strict   (ti'=1,ti=1) strict
so  S0 = strict@mask0 + incl@mask1,  S1 = strict@mask0 + strict@mask1.
"""

import numpy as np

_B, _T, _D, _C = 64, 256, 768, 141
_NCORES = 8
_BPC = _B // _NCORES  # batches per core
_KD = (2 * _D) // 128  # 12 d-tiles of 128

_CACHE = {}
_CONSTS = {}


def _host_consts():
    if _CONSTS:
        return _CONSTS
    import ml_dtypes
    y = np.arange(_C, dtype=np.float32) + 0.5
    cy = np.empty((128, 2, _C), dtype=np.float32)
    cy[:, 0, :] = y
    cy[:, 1, :] = y * y
    p = np.arange(128)
    tr = np.empty((128, 2, 128), dtype=ml_dtypes.bfloat16)
    tr[:, 0, :] = (p[:, None] > p[None, :]).astype(ml_dtypes.bfloat16)
    tr[:, 1, :] = (p[:, None] >= p[None, :]).astype(ml_dtypes.bfloat16)
    _CONSTS["CY"] = cy
    _CONSTS["TR"] = tr
    return _CONSTS


def _build(delta, bias_inv):
    """Build + compile the SPMD Bass program. delta/bias_inv are length-2
    float tuples baked into the NEFF (zero for this problem's setup)."""
    import concourse.bass as bass
    import concourse.tile as tile
    from concourse import bacc, mybir

    f32, i32 = mybir.dt.float32, mybir.dt.int32
    bf16 = mybir.dt.bfloat16
    nc = bacc.Bacc("TRN2", target_bir_lowering=False, debug=False,
                   num_devices=_NCORES)
    h0_d = nc.dram_tensor("h0", [_BPC, _T, _D], bf16, kind="ExternalInput")
    h1_d = nc.dram_tensor("h1", [_BPC, _T, _D], bf16, kind="ExternalInput")
    # offset-derived (s, q) = (lo+hi, lo*hi) pre-permuted on host to
    # [p, b, ti, k] with t = 2p + ti  (mask test is (y*s - q) > y^2)
    offs_d = nc.dram_tensor("offs", [128, _BPC, 2, 2], f32,
                            kind="ExternalInput")
    a_d = nc.dram_tensor("A", [2 * _D, 2], bf16, kind="ExternalInput")
    cy_d = nc.dram_tensor("CY", [128, 2, _C], f32, kind="ExternalInput")
    tr_d = nc.dram_tensor("TR", [128, 2, 128], bf16, kind="ExternalInput")
    out_d = nc.dram_tensor("out", [2, _BPC, _C], f32, kind="ExternalOutput")

    with tile.TileContext(nc) as tc:
        with (
            tc.tile_pool(name="consts", bufs=1) as consts,
            tc.tile_pool(name="hid", bufs=12) as hid_pool,
            tc.tile_pool(name="work", bufs=4) as work,
            tc.tile_pool(name="gath", bufs=6) as gath_pool,
            tc.tile_pool(name="ps_s", bufs=2, space="PSUM") as ps_s,
            tc.tile_pool(name="ps_g", bufs=4, space="PSUM") as ps_g,
            tc.tile_pool(name="ps_o", bufs=2, space="PSUM") as ps_o,
        ):
            # ---- constants + all offsets (host-built, DMA'd once, FIRST on
            #      the two HWDGE queues so they are not stuck behind the
            #      multi-MB hidden-state stream on the shared SDMA engines) ----
            of = consts.tile([128, _BPC, 2, 2], f32)
            nc.sync.dma_start(of[:], offs_d[:])
            cy = consts.tile([128, 2, _C], f32)
            nc.scalar.dma_start(cy[:], cy_d[:])
            tr = consts.tile([128, 2, 128], bf16)  # [:,0]=strict, [:,1]=incl
            nc.sync.dma_start(tr[:], tr_d[:])
            a_sb = consts.tile([128, _KD, 2], bf16)  # A[k*128+p, n] -> [p,k,n]
            nc.scalar.dma_start(
                a_sb[:], a_d[:].rearrange("(k p) n -> p k n", p=128))
            out_all = consts.tile([2, _BPC, _C], f32)
            use_delta = any(v != 0.0 for v in delta)
            if use_delta:
                dl = consts.tile([128, 2], bf16)
                nc.vector.memset(dl[:, 0:1], float(delta[0]))
                nc.vector.memset(dl[:, 1:2], float(delta[1]))
            if any(v != 0.0 for v in bias_inv):
                bi = consts.tile([2, 1], f32)
                nc.vector.memset(bi[0:1, :], float(bias_inv[0]))
                nc.vector.memset(bi[1:2, :], float(bias_inv[1]))
            else:
                bi = None

            op = mybir.AluOpType
            # process batches in groups: suffix/eq/sel/proj run once per group
            # at N=G*141 (batches side by side) to amortize per-op cost.
            # First/last group of 1 for a faster pipeline ramp and drain.
            groups = [[0], [1, 2], [3, 4], [5, 6], [7]]
            cp_cnt = 0
            for pb, grp in enumerate(groups):
                G = len(grp)
                hts = []
                mk_pair = [work.tile([128, G, _C], bf16, tag=f"mask{ti}",
                                     name=f"mk{pb}_{ti}")
                           for ti in range(2)]
                for par in range(G):
                    b = grp[par]
                    # ---- load hidden; t = 2p + ti interleave keeps the DMA
                    #      source fully contiguous ----
                    h0_t = hid_pool.tile([128, 2, _D], bf16, tag="hid")
                    nc.sync.dma_start(
                        h0_t[:], h0_d[b].rearrange("(p ti) d -> p ti d", ti=2))
                    h1_t = hid_pool.tile([128, 2, _D], bf16, tag="hid")
                    nc.scalar.dma_start(
                        h1_t[:], h1_d[b].rearrange("(p ti) d -> p ti d", ti=2))
                    hts.append((h0_t, h1_t))

                    # ---- mask[t,c] = is_gt(y*s - q, y^2)  (bf16 0/1) ----
                    for ti in range(2):
                        w2 = work.tile([128, _C], f32, tag="w2")
                        nc.vector.tensor_scalar(w2[:], cy[:, 0, :],
                                                of[:, b, ti, 0:1],
                                                of[:, b, ti, 1:2],
                                                op.mult, op.subtract)
                        nc.vector.tensor_tensor(mk_pair[ti][:, par, :], w2[:],
                                                cy[:, 1, :], op.is_gt)

                # ---- suffix counts via matmul (N=G*141), then sel ----
                m0f = mk_pair[0][:].rearrange("p a c -> p (a c)")
                m1f = mk_pair[1][:].rearrange("p a c -> p (a c)")
                s0 = ps_s.tile([128, G * _C], f32, tag="suf", name=f"s0_{pb}")
                nc.tensor.matmul(s0[:], tr[:, 0, :], m0f, start=True, stop=False)
                nc.tensor.matmul(s0[:], tr[:, 1, :], m1f, start=False, stop=True)
                msum = work.tile([128, G, _C], bf16, tag="msum",
                                 name=f"ms{pb}")
                nc.vector.tensor_tensor(msum[:], mk_pair[0][:], mk_pair[1][:],
                                        op.add)
                s1 = ps_s.tile([128, G * _C], f32, tag="suf", name=f"s1_{pb}")
                nc.tensor.matmul(s1[:], tr[:, 0, :],
                                 msum[:].rearrange("p a c -> p (a c)"),
                                 start=True, stop=True)
                sels = []  # sels[ti] = [128, G(parity), C] bf16
                for ti, s in ((0, s0), (1, s1)):
                    eq = work.tile([128, G, _C], bf16, tag="eq",
                                   name=f"eq{pb}_{ti}")
                    nc.vector.tensor_scalar(
                        eq[:].rearrange("p a c -> p (a c)"), s[:], 0.0, None,
                        op.is_equal)
                    sl = work.tile([128, G, _C], bf16, tag="sel",
                                   name=f"sl{pb}_{ti}")
                    nc.vector.tensor_tensor(sl[:], eq[:], mk_pair[ti][:],
                                            op.mult)
                    sels.append(sl)

                # ---- gather: BgathT[d,c] = concat.T @ sel; 3 d-tiles per
                #      PSUM bank; pair layout [128, 3, 2, C] feeds proj ----
                bg = []
                for gi in range(4):
                    bgt = gath_pool.tile([128, 3, G, _C], bf16, tag="bg",
                                         name=f"bg{pb}_{gi}")
                    for par in range(G):
                        g = ps_g.tile([128, 3, _C], f32, tag="g",
                                      name=f"g{pb}_{gi}_{par}")
                        for jj in range(3):
                            mi = gi * 3 + jj
                            half, ds = divmod(mi, 6)
                            ht = hts[par][half]
                            for ti in range(2):
                                nc.tensor.matmul(
                                    g[:, jj, :],
                                    ht[:, ti, ds * 128:(ds + 1) * 128],
                                    sels[ti][:, par, :],
                                    start=(ti == 0), stop=(ti == 1))
                        if cp_cnt % 8 in (0, 3, 6):
                            nc.vector.tensor_copy(bgt[:, :, par, :], g[:])
                        else:
                            nc.scalar.copy(bgt[:, :, par, :], g[:])
                        cp_cnt += 1
                    bg.append(bgt)

                # ---- project: outT[2, GC] = A.T @ BgathT (+ delta*valid) ----
                o_ps = ps_o.tile([2, G, _C], f32, tag="o", name=f"o{pb}")
                o_flat = o_ps[:].rearrange("n a c -> n (a c)")
                n_mm = _KD + (2 if use_delta else 0)
                for mi in range(_KD):
                    nc.tensor.matmul(
                        o_flat, a_sb[:, mi, :],
                        bg[mi // 3][:, mi % 3, :, :].rearrange(
                            "p a c -> p (a c)"),
                        start=(mi == 0), stop=(mi == n_mm - 1))
                if use_delta:
                    for ti in range(2):
                        nc.tensor.matmul(
                            o_flat, dl[:],
                            sels[ti][:].rearrange("p a c -> p (a c)"),
                            start=False, stop=(ti == 1))
                b0 = grp[0]
                if bi is None:
                    nc.vector.tensor_copy(out_all[:, b0:b0 + G, :], o_ps[:])
                else:
                    nc.vector.tensor_scalar(out_all[:, b0:b0 + G, :],
                                            o_ps[:], bi[:, 0:1], None, op.add)

            nc.gpsimd.dma_start(out_d[:], out_all[:])

    nc.compile()
    return nc


def _make_in_maps(hidden0, hidden1, offset_mapping, a_full):
    import ml_dtypes
    consts = _host_consts()
    h0b = np.asarray(hidden0, np.float32).astype(ml_dtypes.bfloat16)
    h1b = np.asarray(hidden1, np.float32).astype(ml_dtypes.bfloat16)
    offs = np.asarray(offset_mapping, dtype=np.int64)
    sq = np.empty(offs.shape, np.float32)  # [..., 0]=lo+hi, [..., 1]=lo*hi
    sq[..., 0] = offs[..., 0] + offs[..., 1]
    sq[..., 1] = offs[..., 0] * offs[..., 1]
    in_maps = []
    for i in range(_NCORES):
        sl = slice(i * _BPC, (i + 1) * _BPC)
        # [b, t, k] -> [p, b, ti, k] with t = 2p + ti
        osh = sq[sl].reshape(_BPC, 128, 2, 2).transpose(1, 0, 2, 3)
        in_maps.append({
            "h0": np.ascontiguousarray(h0b[sl]),
            "h1": np.ascontiguousarray(h1b[sl]),
            "offs": np.ascontiguousarray(osh),
            "A": a_full,
            "CY": consts["CY"],
            "TR": consts["TR"],
        })
    return in_maps


def _fold_weights(W_tok, b_tok, W1, b1, W2, b2):
    import ml_dtypes
    w12 = W1.astype(np.float64) @ W2.astype(np.float64)        # [2D, 2]
    a_full = (W_tok.astype(np.float64) @ w12).astype(ml_dtypes.bfloat16)
    delta = tuple(float(x) for x in (b_tok.astype(np.float64) @ w12))
    bias_inv = tuple(float(x) for x in
                     (b1.astype(np.float64) @ W2.astype(np.float64)
                      + b2.astype(np.float64)))
    return a_full, delta, bias_inv


def kernel(hidden0, hidden1, offset_mapping, W_tok, b_tok, W1, b1, W2, b2,
           hidden_state):
    from concourse.bass_utils import run_bass_kernel_spmd

    a_full, delta, bias_inv = _fold_weights(W_tok, b_tok, W1, b1, W2, b2)
    key = (delta, bias_inv)
    if key not in _CACHE:
        _CACHE[key] = _build(delta, bias_inv)
    nc = _CACHE[key]

    in_maps = _make_in_maps(hidden0, hidden1, offset_mapping, a_full)
    res = run_bass_kernel_spmd(nc, in_maps, core_ids=list(range(_NCORES)))

    out = np.empty((_B, 2, _C), np.float32)
    for i in range(_NCORES):
        out[i * _BPC:(i + 1) * _BPC] = res.results[i]["out"].transpose(1, 0, 2)
    start = np.ascontiguousarray(out[:, 0, :, None])
    end = np.ascontiguousarray(out[:, 1, :, None])
    return start, end, np.asarray(hidden_state)


# revision 29
# speedup vs baseline: 1.0956x; 1.0956x over previous
"""Trainium2 Bass kernel for nn_CharLevelModel (token->char scatter + MLP head).

Math: reference computes
  X  = concat(h0,h1) @ W_tok + b_tok          [B,T,2D]
  tok[b,c] = last token t whose char span [lo,hi) covers c (else -1)
  G  = X[b, tok] (0 where invalid)            [B,C,2D]
  out = (G @ W1 + b1) @ W2 + b2               [B,C,2]

Everything is linear, and row-gather commutes with the per-row linear maps, so
with A = W_tok @ W1 @ W2 ([2D,2]):
  out[b,c,:] = valid * (concat[b,tok[b,c],:] @ A + b_tok@W1@W2) + (b1@W2 + b2)

On device (per core, B/8 batches; matmuls in bf16, accumulate fp32):
  mask[t,c]   = (lo[t] <= c) & (c < hi[t])
              = [ (y-lo)(y-hi) < 0 ]  with y = c+0.5   (exact integer f32 math)
              computed as  is_gt(y*s - q, y^2)  with s = lo+hi, q = lo*hi
  suffix[t,c] = sum_{t'>t} mask[t',c]       (matmul w/ triangular-ones blocks)
  sel[t,c]    = mask * (suffix == 0)        (<=1 one per column c)
  BgathT      = concat.T @ sel   via matmul(lhsT=concat_tile, rhs=sel)  [2D,C]
  outT[2,C]   = A.T @ BgathT     via matmul(lhsT=A_tile, rhs=BgathT_tile)
No transposes needed anywhere; both matmul stages consume natural layouts.
sel is exactly 0/1 (bf16-exact) so the gather matmul reproduces the bf16
hidden values exactly; only the bf16 input/A rounding matters (~1e-3 rel).

Tokens are laid out interleaved across the two 128-partition tiles:
t = 2p + ti, so each SBUF partition p holds tokens (2p, 2p+1) and the DMA from
hidden[b] ([256, 768] row-major) is a single fully-contiguous transfer.
With this order the strict "t' > t" block matrices are:
  (ti'=0,ti=0) strict   (ti'=1,ti=0) inclusive
  (ti'=0,ti=1) strict   (ti'=1,ti=1) strict
so  S0 = strict@mask0 + incl@mask1,  S1 = strict@mask0 + strict@mask1.
"""

import numpy as np

_B, _T, _D, _C = 64, 256, 768, 141
_NCORES = 8
_BPC = _B // _NCORES  # batches per core
_KD = (2 * _D) // 128  # 12 d-tiles of 128

_CACHE = {}
_CONSTS = {}


def _host_consts():
    if _CONSTS:
        return _CONSTS
    import ml_dtypes
    y = np.arange(_C, dtype=np.float32) + 0.5
    cy = np.empty((128, 2, _C), dtype=np.float32)
    cy[:, 0, :] = y
    cy[:, 1, :] = y * y
    p = np.arange(128)
    tr = np.empty((128, 2, 128), dtype=ml_dtypes.bfloat16)
    tr[:, 0, :] = (p[:, None] > p[None, :]).astype(ml_dtypes.bfloat16)
    tr[:, 1, :] = (p[:, None] >= p[None, :]).astype(ml_dtypes.bfloat16)
    _CONSTS["CY"] = cy
    _CONSTS["TR"] = tr
    return _CONSTS


def _build(delta, bias_inv):
    """Build + compile the SPMD Bass program. delta/bias_inv are length-2
    float tuples baked into the NEFF (zero for this problem's setup)."""
    import concourse.bass as bass
    import concourse.tile as tile
    from concourse import bacc, mybir

    f32, i32 = mybir.dt.float32, mybir.dt.int32
    bf16 = mybir.dt.bfloat16
    nc = bacc.Bacc("TRN2", target_bir_lowering=False, debug=False,
                   num_devices=_NCORES)
    h0_d = nc.dram_tensor("h0", [_BPC, _T, _D], bf16, kind="ExternalInput")
    h1_d = nc.dram_tensor("h1", [_BPC, _T, _D], bf16, kind="ExternalInput")
    # offset-derived (s, q) = (lo+hi, lo*hi) pre-permuted on host to
    # [p, b, ti, k] with t = 2p + ti  (mask test is (y*s - q) > y^2)
    offs_d = nc.dram_tensor("offs", [128, _BPC, 2, 2], f32,
                            kind="ExternalInput")
    a_d = nc.dram_tensor("A", [2 * _D, 2], bf16, kind="ExternalInput")
    cy_d = nc.dram_tensor("CY", [128, 2, _C], f32, kind="ExternalInput")
    tr_d = nc.dram_tensor("TR", [128, 2, 128], bf16, kind="ExternalInput")
    out_d = nc.dram_tensor("out", [2, _BPC, _C], f32, kind="ExternalOutput")

    with tile.TileContext(nc) as tc:
        with (
            tc.tile_pool(name="consts", bufs=1) as consts,
            tc.tile_pool(name="hid", bufs=12) as hid_pool,
            tc.tile_pool(name="work", bufs=4) as work,
            tc.tile_pool(name="gath", bufs=6) as gath_pool,
            tc.tile_pool(name="ps_s", bufs=2, space="PSUM") as ps_s,
            tc.tile_pool(name="ps_g", bufs=4, space="PSUM") as ps_g,
            tc.tile_pool(name="ps_o", bufs=2, space="PSUM") as ps_o,
        ):
            # ---- constants + all offsets (host-built, DMA'd once, FIRST on
            #      the two HWDGE queues so they are not stuck behind the
            #      multi-MB hidden-state stream on the shared SDMA engines) ----
            of = consts.tile([128, _BPC, 2, 2], f32)
            nc.sync.dma_start(of[:], offs_d[:])
            cy = consts.tile([128, 2, _C], f32)
            nc.scalar.dma_start(cy[:], cy_d[:])
            tr = consts.tile([128, 2, 128], bf16)  # [:,0]=strict, [:,1]=incl
            nc.sync.dma_start(tr[:], tr_d[:])
            a_sb = consts.tile([128, _KD, 2], bf16)  # A[k*128+p, n] -> [p,k,n]
            nc.scalar.dma_start(
                a_sb[:], a_d[:].rearrange("(k p) n -> p k n", p=128))
            out_all = consts.tile([2, _BPC, _C], f32)
            use_delta = any(v != 0.0 for v in delta)
            if use_delta:
                dl = consts.tile([128, 2], bf16)
                nc.vector.memset(dl[:, 0:1], float(delta[0]))
                nc.vector.memset(dl[:, 1:2], float(delta[1]))
            if any(v != 0.0 for v in bias_inv):
                bi = consts.tile([2, 1], f32)
                nc.vector.memset(bi[0:1, :], float(bias_inv[0]))
                nc.vector.memset(bi[1:2, :], float(bias_inv[1]))
            else:
                bi = None

            op = mybir.AluOpType
            # process batches in groups: suffix/eq/sel/proj run once per group
            # at N=G*141 (batches side by side) to amortize per-op cost.
            # First/last group of 1 for a faster pipeline ramp and drain.
            groups = [[0, 1], [2, 3], [4, 5], [6, 7]]
            cp_cnt = 0
            for pb, grp in enumerate(groups):
                G = len(grp)
                hts = []
                mk_pair = [work.tile([128, G, _C], bf16, tag=f"mask{ti}",
                                     name=f"mk{pb}_{ti}")
                           for ti in range(2)]
                for par in range(G):
                    b = grp[par]
                    # ---- load hidden; t = 2p + ti interleave keeps the DMA
                    #      source fully contiguous ----
                    h0_t = hid_pool.tile([128, 2, _D], bf16, tag="hid")
                    nc.sync.dma_start(
                        h0_t[:], h0_d[b].rearrange("(p ti) d -> p ti d", ti=2))
                    h1_t = hid_pool.tile([128, 2, _D], bf16, tag="hid")
                    nc.scalar.dma_start(
                        h1_t[:], h1_d[b].rearrange("(p ti) d -> p ti d", ti=2))
                    hts.append((h0_t, h1_t))

                    # ---- mask[t,c] = is_gt(y*s - q, y^2)  (bf16 0/1) ----
                    for ti in range(2):
                        w2 = work.tile([128, _C], f32, tag="w2")
                        nc.vector.tensor_scalar(w2[:], cy[:, 0, :],
                                                of[:, b, ti, 0:1],
                                                of[:, b, ti, 1:2],
                                                op.mult, op.subtract)
                        nc.vector.tensor_tensor(mk_pair[ti][:, par, :], w2[:],
                                                cy[:, 1, :], op.is_gt)

                # ---- suffix counts via matmul (N=G*141), then sel ----
                m0f = mk_pair[0][:].rearrange("p a c -> p (a c)")
                m1f = mk_pair[1][:].rearrange("p a c -> p (a c)")
                s0 = ps_s.tile([128, G * _C], f32, tag="suf", name=f"s0_{pb}")
                nc.tensor.matmul(s0[:], tr[:, 0, :], m0f, start=True, stop=False)
                nc.tensor.matmul(s0[:], tr[:, 1, :], m1f, start=False, stop=True)
                msum = work.tile([128, G, _C], bf16, tag="msum",
                                 name=f"ms{pb}")
                nc.vector.tensor_tensor(msum[:], mk_pair[0][:], mk_pair[1][:],
                                        op.add)
                s1 = ps_s.tile([128, G * _C], f32, tag="suf", name=f"s1_{pb}")
                nc.tensor.matmul(s1[:], tr[:, 0, :],
                                 msum[:].rearrange("p a c -> p (a c)"),
                                 start=True, stop=True)
                sels = []  # sels[ti] = [128, G(parity), C] bf16
                for ti, s in ((0, s0), (1, s1)):
                    eq = work.tile([128, G, _C], bf16, tag="eq",
                                   name=f"eq{pb}_{ti}")
                    nc.vector.tensor_scalar(
                        eq[:].rearrange("p a c -> p (a c)"), s[:], 0.0, None,
                        op.is_equal)
                    sl = work.tile([128, G, _C], bf16, tag="sel",
                                   name=f"sl{pb}_{ti}")
                    nc.vector.tensor_tensor(sl[:], eq[:], mk_pair[ti][:],
                                            op.mult)
                    sels.append(sl)

                # ---- gather: BgathT[d,c] = concat.T @ sel; 3 d-tiles per
                #      PSUM bank; pair layout [128, 3, 2, C] feeds proj ----
                bg = []
                for gi in range(4):
                    bgt = gath_pool.tile([128, 3, G, _C], bf16, tag="bg",
                                         name=f"bg{pb}_{gi}")
                    for par in range(G):
                        g = ps_g.tile([128, 3, _C], f32, tag="g",
                                      name=f"g{pb}_{gi}_{par}")
                        for jj in range(3):
                            mi = gi * 3 + jj
                            half, ds = divmod(mi, 6)
                            ht = hts[par][half]
                            for ti in range(2):
                                nc.tensor.matmul(
                                    g[:, jj, :],
                                    ht[:, ti, ds * 128:(ds + 1) * 128],
                                    sels[ti][:, par, :],
                                    start=(ti == 0), stop=(ti == 1))
                        if cp_cnt % 8 in (0, 3, 6):
                            nc.vector.tensor_copy(bgt[:, :, par, :], g[:])
                        else:
                            nc.scalar.copy(bgt[:, :, par, :], g[:])
                        cp_cnt += 1
                    bg.append(bgt)

                # ---- project: outT[2, GC] = A.T @ BgathT (+ delta*valid) ----
                o_ps = ps_o.tile([2, G, _C], f32, tag="o", name=f"o{pb}")
                o_flat = o_ps[:].rearrange("n a c -> n (a c)")
                n_mm = _KD + (2 if use_delta else 0)
                for mi in range(_KD):
                    nc.tensor.matmul(
                        o_flat, a_sb[:, mi, :],
                        bg[mi // 3][:, mi % 3, :, :].rearrange(
                            "p a c -> p (a c)"),
                        start=(mi == 0), stop=(mi == n_mm - 1))
                if use_delta:
                    for ti in range(2):
                        nc.tensor.matmul(
                            o_flat, dl[:],
                            sels[ti][:].rearrange("p a c -> p (a c)"),
                            start=False, stop=(ti == 1))
                b0 = grp[0]
                if bi is None:
                    nc.vector.tensor_copy(out_all[:, b0:b0 + G, :], o_ps[:])
                else:
                    nc.vector.tensor_scalar(out_all[:, b0:b0 + G, :],
                                            o_ps[:], bi[:, 0:1], None, op.add)

            nc.gpsimd.dma_start(out_d[:], out_all[:])

    nc.compile()
    return nc


def _make_in_maps(hidden0, hidden1, offset_mapping, a_full):
    import ml_dtypes
    consts = _host_consts()
    h0b = np.asarray(hidden0, np.float32).astype(ml_dtypes.bfloat16)
    h1b = np.asarray(hidden1, np.float32).astype(ml_dtypes.bfloat16)
    offs = np.asarray(offset_mapping, dtype=np.int64)
    sq = np.empty(offs.shape, np.float32)  # [..., 0]=lo+hi, [..., 1]=lo*hi
    sq[..., 0] = offs[..., 0] + offs[..., 1]
    sq[..., 1] = offs[..., 0] * offs[..., 1]
    in_maps = []
    for i in range(_NCORES):
        sl = slice(i * _BPC, (i + 1) * _BPC)
        # [b, t, k] -> [p, b, ti, k] with t = 2p + ti
        osh = sq[sl].reshape(_BPC, 128, 2, 2).transpose(1, 0, 2, 3)
        in_maps.append({
            "h0": np.ascontiguousarray(h0b[sl]),
            "h1": np.ascontiguousarray(h1b[sl]),
            "offs": np.ascontiguousarray(osh),
            "A": a_full,
            "CY": consts["CY"],
            "TR": consts["TR"],
        })
    return in_maps


def _fold_weights(W_tok, b_tok, W1, b1, W2, b2):
    import ml_dtypes
    w12 = W1.astype(np.float64) @ W2.astype(np.float64)        # [2D, 2]
    a_full = (W_tok.astype(np.float64) @ w12).astype(ml_dtypes.bfloat16)
    delta = tuple(float(x) for x in (b_tok.astype(np.float64) @ w12))
    bias_inv = tuple(float(x) for x in
                     (b1.astype(np.float64) @ W2.astype(np.float64)
                      + b2.astype(np.float64)))
    return a_full, delta, bias_inv


def kernel(hidden0, hidden1, offset_mapping, W_tok, b_tok, W1, b1, W2, b2,
           hidden_state):
    from concourse.bass_utils import run_bass_kernel_spmd

    a_full, delta, bias_inv = _fold_weights(W_tok, b_tok, W1, b1, W2, b2)
    key = (delta, bias_inv)
    if key not in _CACHE:
        _CACHE[key] = _build(delta, bias_inv)
    nc = _CACHE[key]

    in_maps = _make_in_maps(hidden0, hidden1, offset_mapping, a_full)
    res = run_bass_kernel_spmd(nc, in_maps, core_ids=list(range(_NCORES)))

    out = np.empty((_B, 2, _C), np.float32)
    for i in range(_NCORES):
        out[i * _BPC:(i + 1) * _BPC] = res.results[i]["out"].transpose(1, 0, 2)
    start = np.ascontiguousarray(out[:, 0, :, None])
    end = np.ascontiguousarray(out[:, 1, :, None])
    return start, end, np.asarray(hidden_state)
